# revision 11
# baseline (speedup 1.0000x reference)
"""Trainium2 Bass kernel for the 10-mode gate contraction.

Computes y = transpose_back(einsum('ab...,ABab->AB...', transpose(x), B))
for x of shape (6,)*10, gate wires [2, 5], B of shape (6, 6, 6, 6).

Viewing x (C-contiguous) as [P=36, a=6, Q=36, b=6, R=1296] (P = modes 0-1,
a = mode 2, Q = modes 3-4, b = mode 5, R = modes 6-9), the op is a batched
36x36 matmul over the (a, b) legs.  The host re-lays x out as
[a, b, P, Q, r] in float16 and shards the last mode R across the 8 cores
(162 columns each); each core then applies the gate with a block-diagonal
108x108 stationary operand (3 independent P-slices per matmul, using 108 of
the 128 PE partitions), so both permutes are absorbed into the DMA access
patterns.  fp16 halves the HBM traffic (the roofline) and runs the PE at
1 cycle/row instead of fp32's 4; PSUM accumulation stays fp32.
"""

import sys

sys.path.insert(0, "/opt/trn_rl_repo")

import numpy as np

NCORES = 8
C = 6                       # cutoff
A6, B6, NP, NQ, NR = 6, 6, 36, 36, 1296
RS = NR // NCORES           # 162 columns of R per core
QR = NQ * RS                # 5832 free elements per P-slice
NCHUNK = 486                # matmul free-dim chunk (12 per tile, <= 512 f32)
GROUP = 3                   # P-values packed per matmul via block-diag weights

_compiled = None


def _install_axon_hooks_shim():
    """bass_utils's trace path imports antenv.axon_hooks, which this image's
    antenv package lacks.  Provide it (ctypes hook into libaxon_pjrt.so when
    present, else None so bass_utils degrades to no-trace) so running with
    BASS_TRACE=1 works instead of crashing."""
    if "antenv.axon_hooks" in sys.modules:
        return
    import contextlib
    import ctypes
    import types

    so_path = "/opt/axon/libaxon_pjrt.so"
    hook = None
    try:
        lib = ctypes.CDLL(so_path)
        if hasattr(lib, "axon_start_nrt_profile"):
            lib.axon_start_nrt_profile.argtypes = [
                ctypes.POINTER(ctypes.c_int64),
                ctypes.c_size_t,
            ]
            lib.axon_start_nrt_profile.restype = ctypes.c_int64
            lib.axon_stop_nrt_profile.argtypes = [ctypes.c_char_p]
            lib.axon_stop_nrt_profile.restype = ctypes.c_int64

            @contextlib.contextmanager
            def hook(output_dir, device_ids):
                import jax

                jax.devices()
                if device_ids:
                    ids = (ctypes.c_int64 * len(device_ids))(*device_ids)
                    rc = lib.axon_start_nrt_profile(ids, len(device_ids))
                else:
                    rc = lib.axon_start_nrt_profile(None, 0)
                if rc != 0:
                    raise RuntimeError(f"axon_start_nrt_profile rc={rc}")
                try:
                    yield
                finally:
                    n = lib.axon_stop_nrt_profile(str(output_dir).encode())
                    if n < 0:
                        raise RuntimeError(f"axon_stop_nrt_profile rc={n}")
    except OSError:
        hook = None

    shim = types.ModuleType("antenv.axon_hooks")
    shim.get_axon_ntff_profile_hook = lambda: hook
    shim.set_axon_ntff_profile_hook = lambda h: None
    sys.modules["antenv.axon_hooks"] = shim


def _build():
    global _compiled
    if _compiled is not None:
        return _compiled

    import concourse.bacc as bacc
    import concourse.mybir as mybir
    import concourse.tile as tile

    DT = mybir.dt.float16
    PT = mybir.dt.float32
    nc = bacc.Bacc("TRN2", target_bir_lowering=False, debug=False,
                   num_devices=NCORES)
    x_in = nc.dram_tensor("x", [A6, B6, NP, NQ, RS], DT, kind="ExternalInput")
    w_in = nc.dram_tensor("w", [108, 108], DT, kind="ExternalInput")
    y_out = nc.dram_tensor("y", [A6, B6, NP, NQ, RS], DT, kind="ExternalOutput")

    with tile.TileContext(nc) as tc:
        with (
            tc.tile_pool(name="wpool", bufs=1) as wpool,
            tc.tile_pool(name="inpool", bufs=4) as inpool,
            tc.tile_pool(name="outpool", bufs=4) as outpool,
            tc.tile_pool(name="psum", bufs=8, space="PSUM") as psum_pool,
        ):
            wtile = wpool.tile([108, 108], DT)
            # weights go via the scalar HWDGE ring so the sync queue's first
            # dispatch is already the first x tile
            nc.scalar.dma_start(out=wtile[:, :], in_=w_in.ap())

            HALF = QR // 2
            NTILES = NP // GROUP
            for ti, p0 in enumerate(range(0, NP, GROUP)):
                xt = inpool.tile([108, QR], DT)
                src = x_in.ap()[:, :, p0:p0 + GROUP].rearrange(
                    "a b g q r -> a b g (q r)")
                # split-tile DMAs: matmuls on the first piece start while the
                # rest is still in flight
                in_step = HALF
                for c0 in range(0, QR, in_step):
                    nc.sync.dma_start(out=xt[:, c0:c0 + in_step],
                                      in_=src[:, :, :, c0:c0 + in_step])

                out_step = HALF
                ot = outpool.tile([108, QR], DT)
                dst = y_out.ap()[:, :, p0:p0 + GROUP].rearrange(
                    "a b g q r -> a b g (q r)")
                for i, c in enumerate(range(0, QR, NCHUNK)):
                    ps = psum_pool.tile([108, NCHUNK], PT)
                    nc.tensor.matmul(out=ps[:, :], lhsT=wtile[:, :],
                                     rhs=xt[:, c:c + NCHUNK],
                                     start=True, stop=True)
                    # PSUM->SBUF cast-copies alternate vector/scalar so
                    # neither engine becomes the pipeline bottleneck
                    if i % 2 == 0:
                        nc.vector.tensor_copy(out=ot[:, c:c + NCHUNK],
                                              in_=ps[:, :])
                    else:
                        nc.scalar.copy(out=ot[:, c:c + NCHUNK], in_=ps[:, :])
                    # out-DMAs issue away from the sync queue so they never
                    # block input-DMA dispatch; alternate scalar's HWDGE
                    # ring and gpsimd's SWDGE ring to halve each queue's
                    # dispatch chain
                    end = c + NCHUNK
                    if end % out_step == 0:
                        c0 = end - out_step
                        eng = nc.scalar if end == HALF else nc.gpsimd
                        eng.dma_start(out=dst[:, :, :, c0:end],
                                      in_=ot[:, c0:end])

    nc.compile()
    _compiled = nc
    return nc


# x modes -> [a, b, P0, P1, Q0, Q1, R...] permutation and its inverse
_PERM = (2, 5, 0, 1, 3, 4, 6, 7, 8, 9)
_INV_PERM = (2, 3, 0, 4, 5, 1, 6, 7, 8, 9)


def _prep_weights(B):
    Bm = np.ascontiguousarray(np.asarray(B), dtype=np.float32).reshape(36, 36)
    W = np.zeros((108, 108), np.float16)
    W4 = W.reshape(36, GROUP, 36, GROUP)
    BmT = Bm.T.astype(np.float16)
    for g in range(GROUP):
        W4[:, g, :, g] = BmT
    return W


def _run(x, B, trace=False, **kwargs):
    _install_axon_hooks_shim()
    from concourse.bass_utils import run_bass_kernel_spmd

    nc = _build()
    W = _prep_weights(B)
    xv = np.asarray(x).reshape((C,) * 10).transpose(_PERM).reshape(
        A6, B6, NP, NQ, NR).astype(np.float16)
    in_maps = [
        {"x": np.ascontiguousarray(xv[..., k * RS:(k + 1) * RS]), "w": W}
        for k in range(NCORES)
    ]
    res = run_bass_kernel_spmd(nc, in_maps, list(range(NCORES)),
                               trace=trace, **kwargs)
    yp = np.empty((A6, B6, NP, NQ, NR), np.float32)
    for k in range(NCORES):
        yp[..., k * RS:(k + 1) * RS] = res.results[k]["y"].astype(np.float32)
    y = np.ascontiguousarray(
        yp.reshape((C,) * 10).transpose(_INV_PERM))
    return y, res


def kernel(x, B):
    y, _ = _run(x, B)
    return y


# revision 13
# speedup vs baseline: 1.0230x; 1.0230x over previous
"""Trainium2 Bass kernel for the 10-mode gate contraction.

Computes y = transpose_back(einsum('ab...,ABab->AB...', transpose(x), B))
for x of shape (6,)*10, gate wires [2, 5], B of shape (6, 6, 6, 6).

Viewing x (C-contiguous) as [P=36, a=6, Q=36, b=6, R=1296] (P = modes 0-1,
a = mode 2, Q = modes 3-4, b = mode 5, R = modes 6-9), the op is a batched
36x36 matmul over the (a, b) legs.  The host re-lays x out as
[a, b, P, Q, r] in float16 and shards the last mode R across the 8 cores
(162 columns each); each core then applies the gate with a block-diagonal
108x108 stationary operand (3 independent P-slices per matmul, using 108 of
the 128 PE partitions), so both permutes are absorbed into the DMA access
patterns.  fp16 halves the HBM traffic (the roofline) and runs the PE at
1 cycle/row instead of fp32's 4; PSUM accumulation stays fp32.
"""

import sys

sys.path.insert(0, "/opt/trn_rl_repo")

import numpy as np

NCORES = 8
C = 6                       # cutoff
A6, B6, NP, NQ, NR = 6, 6, 36, 36, 1296
RS = NR // NCORES           # 162 columns of R per core
QR = NQ * RS                # 5832 free elements per P-slice
NCHUNK = 486                # matmul free-dim chunk (12 per tile, <= 512 f32)
GROUP = 3                   # P-values packed per matmul via block-diag weights

_compiled = None


def _install_axon_hooks_shim():
    """bass_utils's trace path imports antenv.axon_hooks, which this image's
    antenv package lacks.  Provide it (ctypes hook into libaxon_pjrt.so when
    present, else None so bass_utils degrades to no-trace) so running with
    BASS_TRACE=1 works instead of crashing."""
    if "antenv.axon_hooks" in sys.modules:
        return
    import contextlib
    import ctypes
    import types

    so_path = "/opt/axon/libaxon_pjrt.so"
    hook = None
    try:
        lib = ctypes.CDLL(so_path)
        if hasattr(lib, "axon_start_nrt_profile"):
            lib.axon_start_nrt_profile.argtypes = [
                ctypes.POINTER(ctypes.c_int64),
                ctypes.c_size_t,
            ]
            lib.axon_start_nrt_profile.restype = ctypes.c_int64
            lib.axon_stop_nrt_profile.argtypes = [ctypes.c_char_p]
            lib.axon_stop_nrt_profile.restype = ctypes.c_int64

            @contextlib.contextmanager
            def hook(output_dir, device_ids):
                import jax

                jax.devices()
                if device_ids:
                    ids = (ctypes.c_int64 * len(device_ids))(*device_ids)
                    rc = lib.axon_start_nrt_profile(ids, len(device_ids))
                else:
                    rc = lib.axon_start_nrt_profile(None, 0)
                if rc != 0:
                    raise RuntimeError(f"axon_start_nrt_profile rc={rc}")
                try:
                    yield
                finally:
                    n = lib.axon_stop_nrt_profile(str(output_dir).encode())
                    if n < 0:
                        raise RuntimeError(f"axon_stop_nrt_profile rc={n}")
    except OSError:
        hook = None

    shim = types.ModuleType("antenv.axon_hooks")
    shim.get_axon_ntff_profile_hook = lambda: hook
    shim.set_axon_ntff_profile_hook = lambda h: None
    sys.modules["antenv.axon_hooks"] = shim


def _build():
    global _compiled
    if _compiled is not None:
        return _compiled

    import concourse.bacc as bacc
    import concourse.mybir as mybir
    import concourse.tile as tile

    DT = mybir.dt.float16
    PT = mybir.dt.float32
    nc = bacc.Bacc("TRN2", target_bir_lowering=False, debug=False,
                   num_devices=NCORES)
    x_in = nc.dram_tensor("x", [A6, B6, NP, NQ, RS], DT, kind="ExternalInput")
    w_in = nc.dram_tensor("w", [108, 108], DT, kind="ExternalInput")
    y_out = nc.dram_tensor("y", [A6, B6, NP, NQ, RS], DT, kind="ExternalOutput")

    with tile.TileContext(nc) as tc:
        with (
            tc.tile_pool(name="wpool", bufs=1) as wpool,
            tc.tile_pool(name="inpool", bufs=6) as inpool,
            tc.tile_pool(name="outpool", bufs=6) as outpool,
            tc.tile_pool(name="psum", bufs=8, space="PSUM") as psum_pool,
        ):
            wtile = wpool.tile([108, 108], DT)
            # weights go via the scalar HWDGE ring so the sync queue's first
            # dispatch is already the first x tile
            nc.scalar.dma_start(out=wtile[:, :], in_=w_in.ap())

            HALF = QR // 2
            NTILES = NP // GROUP
            for ti, p0 in enumerate(range(0, NP, GROUP)):
                xt = inpool.tile([108, QR], DT)
                src = x_in.ap()[:, :, p0:p0 + GROUP].rearrange(
                    "a b g q r -> a b g (q r)")
                # split-tile DMAs: matmuls on the first piece start while the
                # rest is still in flight
                in_step = HALF
                for c0 in range(0, QR, in_step):
                    nc.sync.dma_start(out=xt[:, c0:c0 + in_step],
                                      in_=src[:, :, :, c0:c0 + in_step])

                out_step = HALF
                ot = outpool.tile([108, QR], DT)
                dst = y_out.ap()[:, :, p0:p0 + GROUP].rearrange(
                    "a b g q r -> a b g (q r)")
                for i, c in enumerate(range(0, QR, NCHUNK)):
                    ps = psum_pool.tile([108, NCHUNK], PT)
                    nc.tensor.matmul(out=ps[:, :], lhsT=wtile[:, :],
                                     rhs=xt[:, c:c + NCHUNK],
                                     start=True, stop=True)
                    # PSUM->SBUF cast-copies alternate vector/scalar so
                    # neither engine becomes the pipeline bottleneck
                    if i % 2 == 0:
                        nc.vector.tensor_copy(out=ot[:, c:c + NCHUNK],
                                              in_=ps[:, :])
                    else:
                        nc.scalar.copy(out=ot[:, c:c + NCHUNK], in_=ps[:, :])
                    # out-DMAs issue from the gpsimd queue so they never
                    # block input-DMA dispatch on the sync queue
                    end = c + NCHUNK
                    if end % out_step == 0:
                        c0 = end - out_step
                        nc.gpsimd.dma_start(out=dst[:, :, :, c0:end],
                                            in_=ot[:, c0:end])

    nc.compile()
    _compiled = nc
    return nc


# x modes -> [a, b, P0, P1, Q0, Q1, R...] permutation and its inverse
_PERM = (2, 5, 0, 1, 3, 4, 6, 7, 8, 9)
_INV_PERM = (2, 3, 0, 4, 5, 1, 6, 7, 8, 9)


def _prep_weights(B):
    Bm = np.ascontiguousarray(np.asarray(B), dtype=np.float32).reshape(36, 36)
    W = np.zeros((108, 108), np.float16)
    W4 = W.reshape(36, GROUP, 36, GROUP)
    BmT = Bm.T.astype(np.float16)
    for g in range(GROUP):
        W4[:, g, :, g] = BmT
    return W


def _run(x, B, trace=False, **kwargs):
    _install_axon_hooks_shim()
    from concourse.bass_utils import run_bass_kernel_spmd

    nc = _build()
    W = _prep_weights(B)
    xv = np.asarray(x).reshape((C,) * 10).transpose(_PERM).reshape(
        A6, B6, NP, NQ, NR).astype(np.float16)
    in_maps = [
        {"x": np.ascontiguousarray(xv[..., k * RS:(k + 1) * RS]), "w": W}
        for k in range(NCORES)
    ]
    res = run_bass_kernel_spmd(nc, in_maps, list(range(NCORES)),
                               trace=trace, **kwargs)
    yp = np.empty((A6, B6, NP, NQ, NR), np.float32)
    for k in range(NCORES):
        yp[..., k * RS:(k + 1) * RS] = res.results[k]["y"].astype(np.float32)
    y = np.ascontiguousarray(
        yp.reshape((C,) * 10).transpose(_INV_PERM))
    return y, res


def kernel(x, B):
    y, _ = _run(x, B)
    return y


# revision 15
# speedup vs baseline: 1.1294x; 1.1040x over previous
"""Trainium2 Bass kernel for the 10-mode gate contraction.

Computes y = transpose_back(einsum('ab...,ABab->AB...', transpose(x), B))
for x of shape (6,)*10, gate wires [2, 5], B of shape (6, 6, 6, 6).

Viewing x (C-contiguous) as [P=36, a=6, Q=36, b=6, R=1296] (P = modes 0-1,
a = mode 2, Q = modes 3-4, b = mode 5, R = modes 6-9), the op is a batched
36x36 matmul over the (a, b) legs.  The host re-lays x out as
[a, b, P, Q, r] in float16 and shards the last mode R across the 8 cores
(162 columns each); each core then applies the gate with a block-diagonal
108x108 stationary operand (3 independent P-slices per matmul, using 108 of
the 128 PE partitions), so both permutes are absorbed into the DMA access
patterns.  fp16 halves the HBM traffic (the roofline) and runs the PE at
1 cycle/row instead of fp32's 4; PSUM accumulation stays fp32 (measured
rel err ~6.5e-4).

Pipeline layout (per core, all 12 P-group tiles):
  - input half-tiles DMA on the sync HWDGE queue (weights on scalar's),
  - 12 matmuls per tile against the resident 108x108 block-diag weights,
  - PSUM->SBUF fp16 cast-copies alternate vector/scalar engines,
  - output half-tiles DMA on the gpsimd SWDGE queue.
Keeping the in- and out-streams on separate issue queues and the copies on
two engines makes HBM bandwidth (~358 GB/s/core) the only bottleneck:
steady-state DMA measures ~355-375 GB/s, ~103 us total vs the ~97 us
practical floor (84.5 us of traffic + ~9 us fixed runtime preamble).
"""

import sys

sys.path.insert(0, "/opt/trn_rl_repo")

import numpy as np

NCORES = 8
C = 6                       # cutoff
A6, B6, NP, NQ, NR = 6, 6, 36, 36, 1296
RS = NR // NCORES           # 162 columns of R per core
QR = NQ * RS                # 5832 free elements per P-slice
NCHUNK = 486                # matmul free-dim chunk (12 per tile, <= 512 f32)
GROUP = 3                   # P-values packed per matmul via block-diag weights

_compiled = None


def _install_axon_hooks_shim():
    """bass_utils's trace path imports antenv.axon_hooks, which this image's
    antenv package lacks.  Provide it (ctypes hook into libaxon_pjrt.so when
    present, else None so bass_utils degrades to no-trace) so running with
    BASS_TRACE=1 works instead of crashing."""
    if "antenv.axon_hooks" in sys.modules:
        return
    import contextlib
    import ctypes
    import types

    so_path = "/opt/axon/libaxon_pjrt.so"
    hook = None
    try:
        lib = ctypes.CDLL(so_path)
        if hasattr(lib, "axon_start_nrt_profile"):
            lib.axon_start_nrt_profile.argtypes = [
                ctypes.POINTER(ctypes.c_int64),
                ctypes.c_size_t,
            ]
            lib.axon_start_nrt_profile.restype = ctypes.c_int64
            lib.axon_stop_nrt_profile.argtypes = [ctypes.c_char_p]
            lib.axon_stop_nrt_profile.restype = ctypes.c_int64

            @contextlib.contextmanager
            def hook(output_dir, device_ids):
                import jax

                jax.devices()
                if device_ids:
                    ids = (ctypes.c_int64 * len(device_ids))(*device_ids)
                    rc = lib.axon_start_nrt_profile(ids, len(device_ids))
                else:
                    rc = lib.axon_start_nrt_profile(None, 0)
                if rc != 0:
                    raise RuntimeError(f"axon_start_nrt_profile rc={rc}")
                try:
                    yield
                finally:
                    n = lib.axon_stop_nrt_profile(str(output_dir).encode())
                    if n < 0:
                        raise RuntimeError(f"axon_stop_nrt_profile rc={n}")
    except OSError:
        hook = None

    shim = types.ModuleType("antenv.axon_hooks")
    shim.get_axon_ntff_profile_hook = lambda: hook
    shim.set_axon_ntff_profile_hook = lambda h: None
    sys.modules["antenv.axon_hooks"] = shim


def _build():
    global _compiled
    if _compiled is not None:
        return _compiled

    import concourse.bacc as bacc
    import concourse.mybir as mybir
    import concourse.tile as tile

    DT = mybir.dt.float16
    PT = mybir.dt.float32
    nc = bacc.Bacc("TRN2", target_bir_lowering=False, debug=False,
                   num_devices=NCORES)
    x_in = nc.dram_tensor("x", [A6, B6, NP, NQ, RS], DT, kind="ExternalInput")
    w_in = nc.dram_tensor("w", [108, 108], DT, kind="ExternalInput")
    y_out = nc.dram_tensor("y", [A6, B6, NP, NQ, RS], DT, kind="ExternalOutput")

    with tile.TileContext(nc) as tc:
        with (
            tc.tile_pool(name="wpool", bufs=1) as wpool,
            tc.tile_pool(name="inpool", bufs=4) as inpool,
            tc.tile_pool(name="outpool", bufs=4) as outpool,
            tc.tile_pool(name="psum", bufs=8, space="PSUM") as psum_pool,
        ):
            wtile = wpool.tile([108, 108], DT)
            # weights go via the scalar HWDGE ring so the sync queue's first
            # dispatch is already the first x tile
            nc.scalar.dma_start(out=wtile[:, :], in_=w_in.ap())

            HALF = QR // 2
            NTILES = NP // GROUP
            for ti, p0 in enumerate(range(0, NP, GROUP)):
                xt = inpool.tile([108, QR], DT)
                src = x_in.ap()[:, :, p0:p0 + GROUP].rearrange(
                    "a b g q r -> a b g (q r)")
                # split-tile DMAs: matmuls on the first piece start while the
                # rest is still in flight
                in_step = HALF
                for c0 in range(0, QR, in_step):
                    nc.sync.dma_start(out=xt[:, c0:c0 + in_step],
                                      in_=src[:, :, :, c0:c0 + in_step])

                out_step = HALF
                ot = outpool.tile([108, QR], DT)
                dst = y_out.ap()[:, :, p0:p0 + GROUP].rearrange(
                    "a b g q r -> a b g (q r)")
                for i, c in enumerate(range(0, QR, NCHUNK)):
                    ps = psum_pool.tile([108, NCHUNK], PT)
                    nc.tensor.matmul(out=ps[:, :], lhsT=wtile[:, :],
                                     rhs=xt[:, c:c + NCHUNK],
                                     start=True, stop=True)
                    # PSUM->SBUF cast-copies alternate vector/scalar so
                    # neither engine becomes the pipeline bottleneck
                    if i % 2 == 0:
                        nc.vector.tensor_copy(out=ot[:, c:c + NCHUNK],
                                              in_=ps[:, :])
                    else:
                        nc.scalar.copy(out=ot[:, c:c + NCHUNK], in_=ps[:, :])
                    # out-DMAs issue from the gpsimd queue so they never
                    # block input-DMA dispatch on the sync queue
                    end = c + NCHUNK
                    if end % out_step == 0:
                        c0 = end - out_step
                        nc.gpsimd.dma_start(out=dst[:, :, :, c0:end],
                                            in_=ot[:, c0:end])

    nc.compile()
    _compiled = nc
    return nc


# x modes -> [a, b, P0, P1, Q0, Q1, R...] permutation and its inverse
_PERM = (2, 5, 0, 1, 3, 4, 6, 7, 8, 9)
_INV_PERM = (2, 3, 0, 4, 5, 1, 6, 7, 8, 9)


def _prep_weights(B):
    Bm = np.ascontiguousarray(np.asarray(B), dtype=np.float32).reshape(36, 36)
    W = np.zeros((108, 108), np.float16)
    W4 = W.reshape(36, GROUP, 36, GROUP)
    BmT = Bm.T.astype(np.float16)
    for g in range(GROUP):
        W4[:, g, :, g] = BmT
    return W


def _run(x, B, trace=False, **kwargs):
    _install_axon_hooks_shim()
    from concourse.bass_utils import run_bass_kernel_spmd

    nc = _build()
    W = _prep_weights(B)
    xv = np.asarray(x).reshape((C,) * 10).transpose(_PERM).reshape(
        A6, B6, NP, NQ, NR).astype(np.float16)
    in_maps = [
        {"x": np.ascontiguousarray(xv[..., k * RS:(k + 1) * RS]), "w": W}
        for k in range(NCORES)
    ]
    res = run_bass_kernel_spmd(nc, in_maps, list(range(NCORES)),
                               trace=trace, **kwargs)
    yp = np.empty((A6, B6, NP, NQ, NR), np.float32)
    for k in range(NCORES):
        yp[..., k * RS:(k + 1) * RS] = res.results[k]["y"].astype(np.float32)
    y = np.ascontiguousarray(
        yp.reshape((C,) * 10).transpose(_INV_PERM))
    return y, res


def kernel(x, B):
    y, _ = _run(x, B)
    return y
